# revision 7
# baseline (speedup 1.0000x reference)
"""CIN (Compressed Interaction Network) Trainium2 kernel.

Reference computation (per batch row b, emb dim d):
    h0 = x                                  [B, 64, 16]
    h_l[b,n,d] = sum_{i,j} x[b,i,d] * h_{l-1}[b,j,d] * Wl[i*Fi+j, n]
    out = concat([sum_d h1, sum_d h2, sum_d h3], axis=1)   [B, 384]

Strategy (pure data parallel over 8 cores, B_loc = 256):
  * Everything lives in "field-major" layout [field, (b,d)] with
    c = b*16+d as the free/column axis (C = 4096 per core).
  * A CIN layer is z[n, c] = sum_(ij) W[(ij), n] * P[(ij), c] where
    P = Khatri-Rao product P[(i,j), c] = X[i,c]*H[j,c], contracted on
    TensorE with PSUM accumulation over 128-row (ij) chunks.
  * Layer 1's P depends only on x, so it is built ON THE HOST
    (symmetrized: 2080 unordered pairs in 17 chunks) and streamed in.
  * Layer 2's chunks are balanced to minimize on-chip replication:
    chunk (g, s), partition p -> (i, j) = (8g + p//16, 16s + p%16).
    The X factor (8 rows x 16 dups per block, 2 MB) is host-replicated
    and streamed; the H factor [128, 8*CB] (16 rows x 8 s-slices) is
    built on-device from h1 with 8 small SBUF->SBUF copies + 3
    partition-doubling DMAs.  One fused bf16 tensor_mul per g builds
    all 8 chunks of that g (H-tile contiguous, X-tile read 8x via a
    stride-0 outer free dim).
  * Layer 3 only needs the d-summed output, so it is restructured as
    out3[b,:] = vec(G2[b]) @ W2 with G2[b,i,j] = sum_d x[b,i,d]*h2[b,j,d],
    computed with PE transposes of h2 + block-diagonal matmuls against
    a host-prepared block-diagonal x tensor.  Layer-3 work is
    interleaved per column block to keep TensorE dense.
  * Columns are processed in four blocks of 1024; the next block's
    layer 1 and H-tile build are interleaved into this block's layer-2
    g-loop so TensorE never waits at block boundaries.  Queue split:
    xp1 on sync+scalar (HWDGE), bulk consts + X tiles + reduces on
    gpsimd, d-sum reduces stay on VectorE (gpsimd cannot reduce X).
"""

import sys

import numpy as np

try:
    import concourse.bass as bass  # noqa: F401
except ImportError:  # grading env fallback
    sys.path.insert(0, "/opt/trn_rl_repo")

import ml_dtypes
import concourse.bacc as bacc
import concourse.bass as bass
import concourse.mybir as mybir
import concourse.tile as tile
from concourse.bass_utils import run_bass_kernel_spmd

BF16 = mybir.dt.bfloat16
F32 = mybir.dt.float32

B, F0, D = 2048, 64, 16
NCORES = 8
BL = B // NCORES          # 256 batch rows per core
C = BL * D                # 4096 columns (b, d)
FN = 128                  # layer width (all three CIN layers)
CT = 512                  # matmul N tile (one PSUM bank of fp32)
CB = 1024                 # column block
NBLK = C // CB            # 4
NCT = CB // CT            # 2 column tiles per block
NG = BL // 8              # 32 groups of 8 batch rows (layer-3 path)
NGB = CB // 128           # 8 layer-3 groups per block
NBH = NBLK // 2           # 2 blocks per layer-3 half
SYM_PAIRS = F0 * (F0 + 1) // 2          # 2080 unordered (i,j) pairs
L1_CHUNKS = (SYM_PAIRS + 127) // 128    # 17 (last chunk zero-padded)
NGRP = 8                  # layer-2 i-groups (8 i-rows each)
NSL = 8                   # layer-2 j-slices (16 j-rows each)

_CACHE = {}


def _build_program():
    nc = bacc.Bacc(None, target_bir_lowering=False)

    xp1_d = nc.dram_tensor("xp1", [NBLK, L1_CHUNKS, 128, CB], BF16, kind="ExternalInput")
    xb_d = nc.dram_tensor("xbt", [NBLK, 128, NGRP * CB], BF16, kind="ExternalInput")
    xdiag_d = nc.dram_tensor("xdiag", [128, NG * 512], BF16, kind="ExternalInput")
    w0_d = nc.dram_tensor("w0c", [128, L1_CHUNKS * FN], BF16, kind="ExternalInput")
    w1_d = nc.dram_tensor("w1c", [128, NGRP * NSL * FN], BF16, kind="ExternalInput")
    w2_d = nc.dram_tensor("w2c", [128, F0 * FN], BF16, kind="ExternalInput")
    ident_d = nc.dram_tensor("ident", [128, 128], BF16, kind="ExternalInput")
    out_d = nc.dram_tensor("out_nb", [3, 128, BL], F32, kind="ExternalOutput")

    with tile.TileContext(nc) as tc:
        with (
            tc.tile_pool(name="const", bufs=1) as const,
            tc.tile_pool(name="hbuf", bufs=1) as hbuf,
            tc.tile_pool(name="outs", bufs=1) as outs,
            tc.tile_pool(name="h1p", bufs=2) as h1p,
            tc.tile_pool(name="ap", bufs=2) as apool,
            tc.tile_pool(name="xbp", bufs=2) as xbp,
            tc.tile_pool(name="p1s", bufs=8) as p1s,
            tc.tile_pool(name="pkr", bufs=2) as pkr,
            tc.tile_pool(name="zp", bufs=4, space="PSUM") as zp,
            tc.tile_pool(name="l3ps", bufs=2, space="PSUM") as l3ps,
            tc.tile_pool(name="o3p", bufs=1, space="PSUM") as o3p,
            tc.tile_pool(name="wps", bufs=1, space="PSUM") as wps,
            tc.tile_pool(name="l3sb", bufs=1) as l3sb,
            tc.tile_pool(name="hts", bufs=6) as hts,
            tc.tile_pool(name="xdg", bufs=6) as xdg,
        ):
            w0_sb = const.tile([128, L1_CHUNKS * FN], BF16)
            nc.scalar.dma_start(w0_sb[:], w0_d[:])
            w1_sb = const.tile([128, NGRP * NSL * FN], BF16)
            w2_sb = const.tile([128, F0 * FN], BF16)
            ident_sb = const.tile([128, 128], BF16)

            h2_sb = hbuf.tile([128, C], BF16, tag="h2")
            out_sb = outs.tile([128, 3 * BL], F32)

            # dense junk-matmul burst at kernel start: pulls the PE HAM
            # clock gate to 8/8 before the real accumulation chains begin.
            warm_sb = const.tile([128, 512], BF16)
            nc.vector.memset(warm_sb[:], 0.0)
            warm_ps = wps.tile([128, CT], F32, tag="w", name="warm_ps")
            for w in range(16):
                nc.tensor.matmul(
                    warm_ps[:],
                    warm_sb[:, 0:128],
                    warm_sb[:],
                    start=(w == 0),
                    stop=(w == 15),
                )

            # first XB stream + the remaining big consts (gpsimd queue so
            # they never head-of-line block the xp1 stream on sync/scalar)
            xb_tiles = {}
            xb_tiles[0] = xbp.tile([128, NGRP * CB], BF16, tag="xb", name="xb_0")
            nc.gpsimd.dma_start(xb_tiles[0][:], xb_d[0])
            nc.gpsimd.dma_start(w1_sb[:], w1_d[:])

            def alloc_z(layer, blk):
                return [
                    zp.tile([128, CT], F32, tag="z", name=f"z{layer}_{blk}_{ct}")
                    for ct in range(NCT)
                ]

            def emit_l1_step(blk, z1, t):
                """One chunk of a block's layer 1: DMA + TensorE only."""
                p1 = p1s.tile([128, CB], BF16, tag="p1", name=f"p1_{blk}_{t}")
                (nc.sync if t % 2 == 0 else nc.scalar).dma_start(p1[:], xp1_d[blk, t])
                for ct in range(NCT):
                    nc.tensor.matmul(
                        z1[ct][:],
                        w0_sb[:, t * FN : (t + 1) * FN],
                        p1[:, ct * CT : (ct + 1) * CT],
                        start=(t == 0),
                        stop=(t == L1_CHUNKS - 1),
                    )

            def emit_h1_copy_and_abuild(blk, z1):
                """z1 -> h1_sb (bf16), the layer-2 H-tile build, and the
                out1 d-reduce (on gpsimd, keeping VectorE free)."""
                h1_sb = h1p.tile([128, CB], BF16, tag="h1", name=f"h1_{blk}")
                for ct in range(NCT):
                    nc.scalar.copy(
                        h1_sb[:, ct * CT : (ct + 1) * CT], z1[ct][:]
                    )
                a_sb = apool.tile([128, NSL * CB], BF16, tag="a", name=f"a_{blk}")
                for s in range(NSL):
                    (nc.sync if s % 2 == 0 else nc.scalar).dma_start(
                        a_sb[0:16, s * CB : (s + 1) * CB],
                        h1_sb[16 * s : 16 * s + 16, :],
                    )
                nc.sync.dma_start(a_sb[16:32, :], a_sb[0:16, :])
                nc.sync.dma_start(a_sb[32:64, :], a_sb[0:32, :])
                nc.scalar.dma_start(a_sb[64:96, :], a_sb[0:32, :])
                nc.scalar.dma_start(a_sb[96:128, :], a_sb[0:32, :])
                bo = blk * (CB // D)
                nc.vector.reduce_sum(
                    out_sb[:, bo : bo + CB // D],
                    h1_sb[:].rearrange("p (b d) -> p b d", d=D),
                    axis=mybir.AxisListType.X,
                )
                nc.sync.dma_start(
                    out_d[0][:, bo : bo + CB // D], out_sb[:, bo : bo + CB // D]
                )
                return a_sb

            # ---- layer 1 of block 0 (DMA-fed, no VectorE) ----
            z1_cur = alloc_z(1, 0)
            for t in range(L1_CHUNKS):
                emit_l1_step(0, z1_cur, t)
            a_cur = emit_h1_copy_and_abuild(0, z1_cur)

            g2t_tiles = {}
            for blk in range(NBLK):
                c0 = blk * CB
                half_idx = blk // NBH
                if blk % NBH == 0:
                    g2t_tiles[half_idx] = l3sb.tile(
                        [128, NBH * NGB * 512],
                        BF16,
                        tag="g2t",
                        name=f"g2t_{half_idx}",
                    )
                a_sb = a_cur
                xb_sb = xb_tiles[blk]

                # ---------------- layer 2 over this block ----------------
                z2 = alloc_z(2, blk)
                if blk + 1 < NBLK:
                    xb_tiles[blk + 1] = xbp.tile(
                        [128, NGRP * CB], BF16, tag="xb", name=f"xb_{blk + 1}"
                    )
                    z1_cur = alloc_z(1, blk + 1)

                l3blk = blk - 1
                for g in range(NGRP):
                    # interleave next block's layer 1 (front-loaded 3:1)
                    if blk + 1 < NBLK:
                        for t in range(3 * g, min(3 * g + 3, L1_CHUNKS)):
                            emit_l1_step(blk + 1, z1_cur, t)
                    # deferred bulk loads: next X tile early, layer-3 consts
                    # mid-block, all on the gpsimd stream queue
                    if blk + 1 < NBLK and g == 1:
                        nc.gpsimd.dma_start(xb_tiles[blk + 1][:], xb_d[blk + 1])
                    if blk == 0 and g == 4:
                        nc.gpsimd.dma_start(ident_sb[:], ident_d[:])
                        nc.gpsimd.dma_start(w2_sb[:], w2_d[:])
                    # one fused TT builds the 8 chunks (g, s) for this g
                    p_sb = pkr.tile(
                        [128, NSL * CB], BF16, tag="p", name=f"p2_{blk}_{g}"
                    )
                    xg = xb_sb[:, g * CB : (g + 1) * CB]
                    nc.vector.tensor_mul(
                        p_sb[:].rearrange("p (s c) -> p s c", s=NSL),
                        a_sb[:].rearrange("p (s c) -> p s c", s=NSL),
                        xg.unsqueeze(1).broadcast_to((128, NSL, CB)),
                    )
                    for s in range(NSL):
                        k = g * NSL + s
                        for ct in range(NCT):
                            nc.tensor.matmul(
                                z2[ct][:],
                                w1_sb[:, k * FN : (k + 1) * FN],
                                p_sb[:, s * CB + ct * CT : s * CB + (ct + 1) * CT],
                                start=(k == 0),
                                stop=(k == NGRP * NSL - 1),
                            )
                    # next block's h1/H-tile as soon as its layer 1 is done
                    if blk + 1 < NBLK and g == 5:
                        a_cur = emit_h1_copy_and_abuild(blk + 1, z1_cur)
                    # interleave layer 3 of the previous block (8 groups)
                    if l3blk >= 0:
                        gl = g
                        hidx = l3blk // NBH
                        g2t_sb = g2t_tiles[hidx]
                        gg = l3blk * NGB + gl
                        gh = (l3blk % NBH) * NGB + gl
                        ht_ps = l3ps.tile(
                            [128, 128], BF16, tag="l3", name=f"htps_{gg}"
                        )
                        nc.tensor.transpose(
                            ht_ps[:],
                            h2_sb[:, gg * 128 : (gg + 1) * 128],
                            ident_sb[:],
                        )
                        ht_sb = hts.tile(
                            [128, 128], BF16, tag="hts", name=f"htsb_{gg}"
                        )
                        nc.scalar.copy(ht_sb[:], ht_ps[:])
                        xd_sb = xdg.tile([128, 512], BF16, tag="xd", name=f"xd_{gg}")
                        nc.gpsimd.dma_start(
                            xd_sb[:], xdiag_d[:, gg * 512 : (gg + 1) * 512]
                        )
                        g2_ps = l3ps.tile(
                            [128, 512], F32, tag="l3", name=f"g2ps_{gg}"
                        )
                        nc.tensor.matmul(g2_ps[:], ht_sb[:], xd_sb[:])
                        nc.scalar.copy(
                            g2t_sb[:, gh * 512 : (gh + 1) * 512], g2_ps[:]
                        )

                # z2 copy-out + out2 reduce (gpsimd) + drain
                for ct in range(NCT):
                    cc = c0 + ct * CT
                    nc.scalar.copy(h2_sb[:, cc : cc + CT], z2[ct][:])
                bo = blk * (CB // D)
                nc.vector.reduce_sum(
                    out_sb[:, BL + bo : BL + bo + CB // D],
                    h2_sb[:, c0 : c0 + CB].rearrange("p (b d) -> p b d", d=D),
                    axis=mybir.AxisListType.X,
                )
                nc.sync.dma_start(
                    out_d[1][:, bo : bo + CB // D],
                    out_sb[:, BL + bo : BL + bo + CB // D],
                )

                # ---- layer-3 contraction pieces (delayed one block) ----
                l3list = [blk - 1] if blk > 0 else []
                if blk == NBLK - 1:
                    l3list.append(blk)
                for l3blk2 in l3list:
                    hidx = l3blk2 // NBH
                    bi = l3blk2 % NBH
                    g2t_sb = g2t_tiles[hidx]
                    if l3blk2 == blk:
                        # emit the last block's 8 L3 groups (no next block
                        # to interleave them into)
                        for gl in range(NGB):
                            gg = l3blk2 * NGB + gl
                            gh = bi * NGB + gl
                            ht_ps = l3ps.tile(
                                [128, 128], BF16, tag="l3", name=f"htps_{gg}"
                            )
                            nc.tensor.transpose(
                                ht_ps[:],
                                h2_sb[:, gg * 128 : (gg + 1) * 128],
                                ident_sb[:],
                            )
                            ht_sb = hts.tile(
                                [128, 128], BF16, tag="hts", name=f"htsb_{gg}"
                            )
                            nc.scalar.copy(ht_sb[:], ht_ps[:])
                            xd_sb = xdg.tile(
                                [128, 512], BF16, tag="xd", name=f"xd_{gg}"
                            )
                            nc.gpsimd.dma_start(
                                xd_sb[:], xdiag_d[:, gg * 512 : (gg + 1) * 512]
                            )
                            g2_ps = l3ps.tile(
                                [128, 512], F32, tag="l3", name=f"g2ps_{gg}"
                            )
                            nc.tensor.matmul(g2_ps[:], ht_sb[:], xd_sb[:])
                            nc.scalar.copy(
                                g2t_sb[:, gh * 512 : (gh + 1) * 512], g2_ps[:]
                            )
                    g2t_r = g2t_sb[:].rearrange(
                        "p (g b i) -> p g b i", b=8, i=F0
                    )
                    if hidx == 1:
                        # last half: contract per block piece (N=64) so the
                        # first piece overlaps the final block's layer 2
                        if bi == 0:
                            o3_last = o3p.tile(
                                [128, 128], F32, tag="o3", name="o3_1"
                            )
                        for i in range(F0):
                            nc.tensor.matmul(
                                o3_last[:, bi * 64 : (bi + 1) * 64],
                                w2_sb[:, i * FN : (i + 1) * FN],
                                g2t_r[:, bi * NGB : (bi + 1) * NGB, :, i],
                                start=(i == 0),
                                stop=(i == F0 - 1),
                            )
                        o3_ps = o3_last
                    elif bi == NBH - 1:
                        # first half: one N=128 chain (fully overlapped)
                        o3_ps = o3p.tile(
                            [128, 128], F32, tag="o3", name=f"o3_{hidx}"
                        )
                        for i in range(F0):
                            nc.tensor.matmul(
                                o3_ps[:],
                                w2_sb[:, i * FN : (i + 1) * FN],
                                g2t_r[:, :, :, i],
                                start=(i == 0),
                                stop=(i == F0 - 1),
                            )
                    if bi == NBH - 1:
                        nc.scalar.copy(
                            out_sb[:, 2 * BL + hidx * 128 : 2 * BL + (hidx + 1) * 128],
                            o3_ps[:],
                        )
                        nc.sync.dma_start(
                            out_d[2][:, hidx * 128 : (hidx + 1) * 128],
                            out_sb[:, 2 * BL + hidx * 128 : 2 * BL + (hidx + 1) * 128],
                        )

    nc.finalize()
    return nc


def _prep_inputs(x, W0, W1, W2):
    """Host-side prep: shard x over cores, transpose/cast, chunk weights,
    build the layer-1 Khatri-Rao product and the layer-2 X-factor tiles."""
    bf = ml_dtypes.bfloat16
    xs = np.ascontiguousarray(x).reshape(NCORES, BL, F0, D)

    # symmetrized layer-1 weights: each unordered pair (i<=j) once, with
    # W0sym[(i,j)] = W0[i*64+j] + W0[j*64+i] (i<j); padded to 17*128 rows
    pi, pj = np.triu_indices(F0)                     # 2080 pairs, i <= j
    W0sym = np.zeros((L1_CHUNKS * 128, FN), dtype=np.float32)
    W0sym[:SYM_PAIRS] = W0[pi * F0 + pj]
    off = W0[pj * F0 + pi].copy()
    off[pi == pj] = 0.0
    W0sym[:SYM_PAIRS] += off
    w0c = (
        W0sym.reshape(L1_CHUNKS, 128, FN)
        .transpose(1, 0, 2)
        .reshape(128, L1_CHUNKS * FN)
    )
    w0c = np.ascontiguousarray(w0c).astype(bf)
    w2c = (
        W2.reshape(F0, 128, FN).transpose(1, 0, 2).reshape(128, F0 * FN)
    )
    w2c = np.ascontiguousarray(w2c).astype(bf)

    # layer-2 chunk (g, s), partition p -> i = 8g + p//16, j = 16s + p%16
    # w1c[p, (g*8+s)*FN + n] = W1[i*128 + j, n]
    W1r = W1.reshape(F0, FN, FN)                     # [i, j, n]
    p_ar = np.arange(128)
    w1c = np.empty((128, NGRP * NSL * FN), dtype=bf)
    for g in range(NGRP):
        for s in range(NSL):
            k = g * NSL + s
            w1c[:, k * FN : (k + 1) * FN] = W1r[
                8 * g + p_ar // 16, 16 * s + p_ar % 16
            ].astype(bf)
    ident = np.eye(128, dtype=np.float32).astype(bf)

    # row -> (i, j) map for the symmetrized layer-1 KR product
    i_idx = np.zeros(L1_CHUNKS * 128, dtype=np.int64)
    j_idx = np.zeros(L1_CHUNKS * 128, dtype=np.int64)
    i_idx[:SYM_PAIRS] = pi
    j_idx[:SYM_PAIRS] = pj

    in_maps = []
    for c in range(NCORES):
        xc = xs[c]                                   # [BL, F0, D]
        xt = xc.transpose(1, 0, 2).reshape(F0, C)    # [i, (b d)]
        xt_bf = xt.astype(bf)
        xt32 = xt_bf.astype(np.float32)

        # host-built layer-1 KR product, bf16-rounded like the device TT
        p1 = (xt32[i_idx] * xt32[j_idx]).astype(bf)  # [17*128, C]
        xp1 = (
            p1.reshape(L1_CHUNKS, 128, NBLK, CB)
            .transpose(2, 0, 1, 3)                   # [blk, t, 128, cb]
            .copy()
        )

        # layer-2 X-factor tiles: xbt[blk, p, g*CB + c] = X[8g + p//16, c]
        xtb = xt_bf.reshape(F0, NBLK, CB)            # [i, blk, cb]
        xbt = (
            xtb.reshape(NGRP, 8, NBLK, CB)[:, np.newaxis, :, :, :]  # g 1 r blk cb
            .repeat(16, axis=1)                      # g dup r blk cb
            .transpose(3, 2, 1, 0, 4)                # blk r dup g cb
            .reshape(NBLK, 128, NGRP * CB)
        )

        # xdiag[(bl', d), (g, bl, i)] = x[g*8+bl, i, d] if bl' == bl else 0
        xd = np.zeros((8, D, NG, 8, F0), dtype=bf)
        xg = xc.reshape(NG, 8, F0, D)                # [g, bl, i, d]
        for bl in range(8):
            xd[bl, :, :, bl, :] = xg[:, bl].transpose(2, 0, 1).astype(bf)
        xdiag = xd.reshape(128, NG * 512)

        in_maps.append(
            {
                "xp1": np.ascontiguousarray(xp1),
                "xbt": np.ascontiguousarray(xbt),
                "xdiag": np.ascontiguousarray(xdiag),
                "w0c": w0c,
                "w1c": np.ascontiguousarray(w1c),
                "w2c": w2c,
                "ident": ident,
            }
        )
    return in_maps


def _postprocess(results):
    # out_nb [3, 128 n, 256 b] per core -> [B, 384]
    outs = [
        np.asarray(r["out_nb"]).transpose(2, 0, 1).reshape(BL, 3 * FN)
        for r in results
    ]
    return np.ascontiguousarray(np.concatenate(outs, axis=0)).astype(np.float32)


def kernel(x, W0, W1, W2, _trace=False, _trace_kwargs=None):
    if "nc" not in _CACHE:
        _CACHE["nc"] = _build_program()
    nc = _CACHE["nc"]
    in_maps = _prep_inputs(
        np.asarray(x, dtype=np.float32),
        np.asarray(W0, dtype=np.float32),
        np.asarray(W1, dtype=np.float32),
        np.asarray(W2, dtype=np.float32),
    )
    kw = {}
    if _trace:
        kw["trace"] = True
        kw.update(_trace_kwargs or {})
    res = run_bass_kernel_spmd(nc, in_maps, core_ids=list(range(NCORES)), **kw)
    out = _postprocess(res.results)
    if _trace:
        _CACHE["last_results"] = res
    return out


# revision 9
# speedup vs baseline: 1.1583x; 1.1583x over previous
"""CIN (Compressed Interaction Network) Trainium2 kernel.

Reference computation (per batch row b, emb dim d):
    h0 = x                                  [B, 64, 16]
    h_l[b,n,d] = sum_{i,j} x[b,i,d] * h_{l-1}[b,j,d] * Wl[i*Fi+j, n]
    out = concat([sum_d h1, sum_d h2, sum_d h3], axis=1)   [B, 384]

Strategy (pure data parallel over 8 cores, B_loc = 256):
  * Everything lives in "field-major" layout [field, (b,d)] with
    c = b*16+d as the free/column axis (C = 4096 per core).
  * A CIN layer is z[n, c] = sum_(ij) W[(ij), n] * P[(ij), c] where
    P = Khatri-Rao product P[(i,j), c] = X[i,c]*H[j,c], contracted on
    TensorE with PSUM accumulation over 128-row (ij) chunks.
  * Layer 1's P depends only on x, so it is built ON THE HOST
    (symmetrized: 2080 unordered pairs in 17 chunks) and streamed in.
  * Layer 2's chunks are balanced to minimize on-chip replication:
    chunk (g, s), partition p -> (i, j) = (8g + p//16, 16s + p%16).
    The X factor (8 rows x 16 dups per block, 2 MB) is host-replicated
    and streamed; the H factor [128, 8*CB] (16 rows x 8 s-slices) is
    built on-device from h1 with 8 small SBUF->SBUF copies + 3
    partition-doubling DMAs.  One fused bf16 tensor_mul per g builds
    all 8 chunks of that g (H-tile contiguous, X-tile read 8x via a
    stride-0 outer free dim).
  * Layer 3 only needs the d-summed output, so it is restructured as
    out3[b,:] = vec(G2[b]) @ W2 with G2[b,i,j] = sum_d x[b,i,d]*h2[b,j,d],
    computed with PE transposes of h2 + block-diagonal matmuls against
    a host-prepared block-diagonal x tensor.  Layer-3 work is
    interleaved per column block to keep TensorE dense.
  * Columns are processed in four blocks of 1024; the next block's
    layer 1 and H-tile build are interleaved into this block's layer-2
    g-loop so TensorE never waits at block boundaries.  Queue split:
    xp1 on sync+scalar (HWDGE), bulk consts + X tiles + reduces on
    gpsimd, d-sum reduces stay on VectorE (gpsimd cannot reduce X).
"""

import sys

import numpy as np

try:
    import concourse.bass as bass  # noqa: F401
except ImportError:  # grading env fallback
    sys.path.insert(0, "/opt/trn_rl_repo")

import ml_dtypes
import concourse.bacc as bacc
import concourse.bass as bass
import concourse.mybir as mybir
import concourse.tile as tile
from concourse.bass_utils import run_bass_kernel_spmd

BF16 = mybir.dt.bfloat16
F32 = mybir.dt.float32

B, F0, D = 2048, 64, 16
NCORES = 8
BL = B // NCORES          # 256 batch rows per core
C = BL * D                # 4096 columns (b, d)
FN = 128                  # layer width (all three CIN layers)
CT = 512                  # matmul N tile (one PSUM bank of fp32)
CB = 1024                 # column block
NBLK = C // CB            # 4
NCT = CB // CT            # 2 column tiles per block
NG = BL // 8              # 32 groups of 8 batch rows (layer-3 path)
NGB = CB // 128           # 8 layer-3 groups per block
NBH = NBLK // 2           # 2 blocks per layer-3 half
SYM_PAIRS = F0 * (F0 + 1) // 2          # 2080 unordered (i,j) pairs
L1_CHUNKS = (SYM_PAIRS + 127) // 128    # 17 (last chunk zero-padded)
NGRP = 8                  # layer-2 i-groups (8 i-rows each)
NSL = 8                   # layer-2 j-slices (16 j-rows each)

_CACHE = {}


def _build_program():
    nc = bacc.Bacc(None, target_bir_lowering=False)

    xp1_d = nc.dram_tensor("xp1", [NBLK, L1_CHUNKS, 128, CB], BF16, kind="ExternalInput")
    xb_d = nc.dram_tensor("xbt", [NBLK, 128, NGRP * CB], BF16, kind="ExternalInput")
    xdiag_d = nc.dram_tensor("xdiag", [128, NG * 512], BF16, kind="ExternalInput")
    w0_d = nc.dram_tensor("w0c", [128, L1_CHUNKS * FN], BF16, kind="ExternalInput")
    w1_d = nc.dram_tensor("w1c", [128, NGRP * NSL * FN], BF16, kind="ExternalInput")
    w2_d = nc.dram_tensor("w2c", [128, F0 * FN], BF16, kind="ExternalInput")
    ident_d = nc.dram_tensor("ident", [128, 128], BF16, kind="ExternalInput")
    out_d = nc.dram_tensor("out_nb", [3, 128, BL], F32, kind="ExternalOutput")

    with tile.TileContext(nc) as tc:
        with (
            tc.tile_pool(name="const", bufs=1) as const,
            tc.tile_pool(name="hbuf", bufs=1) as hbuf,
            tc.tile_pool(name="outs", bufs=1) as outs,
            tc.tile_pool(name="h1p", bufs=2) as h1p,
            tc.tile_pool(name="ap", bufs=2) as apool,
            tc.tile_pool(name="xbp", bufs=2) as xbp,
            tc.tile_pool(name="p1s", bufs=8) as p1s,
            tc.tile_pool(name="pkr", bufs=2) as pkr,
            tc.tile_pool(name="zp", bufs=4, space="PSUM") as zp,
            tc.tile_pool(name="l3ps", bufs=2, space="PSUM") as l3ps,
            tc.tile_pool(name="o3p", bufs=1, space="PSUM") as o3p,
            tc.tile_pool(name="wps", bufs=1, space="PSUM") as wps,
            tc.tile_pool(name="l3sb", bufs=1) as l3sb,
            tc.tile_pool(name="hts", bufs=6) as hts,
            tc.tile_pool(name="xdg", bufs=2) as xdg,
        ):
            w0_sb = const.tile([128, L1_CHUNKS * FN], BF16)
            nc.scalar.dma_start(w0_sb[:], w0_d[:])
            w1_sb = const.tile([128, NGRP * NSL * FN], BF16)
            w2_sb = const.tile([128, F0 * FN], BF16)
            ident_sb = const.tile([128, 128], BF16)

            h2_sb = hbuf.tile([128, C], BF16, tag="h2")
            out_sb = outs.tile([128, 3 * BL], F32)

            # dense junk-matmul burst at kernel start: pulls the PE HAM
            # clock gate to 8/8 before the real accumulation chains begin.
            warm_sb = const.tile([128, 512], BF16)
            nc.vector.memset(warm_sb[:], 0.0)
            warm_ps = wps.tile([128, CT], F32, tag="w", name="warm_ps")
            for w in range(16):
                nc.tensor.matmul(
                    warm_ps[:],
                    warm_sb[:, 0:128],
                    warm_sb[:],
                    start=(w == 0),
                    stop=(w == 15),
                )

            # first XB stream + the remaining big consts (gpsimd queue so
            # they never head-of-line block the xp1 stream on sync/scalar)
            xb_tiles = {}
            xb_tiles[0] = xbp.tile([128, NGRP * CB], BF16, tag="xb", name="xb_0")
            nc.gpsimd.dma_start(xb_tiles[0][:], xb_d[0])
            nc.gpsimd.dma_start(w1_sb[:], w1_d[:])
            nc.gpsimd.dma_start(ident_sb[:], ident_d[:])

            def alloc_z(layer, blk):
                return [
                    zp.tile([128, CT], F32, tag="z", name=f"z{layer}_{blk}_{ct}")
                    for ct in range(NCT)
                ]

            def emit_l1_step(blk, z1, t):
                """One chunk of a block's layer 1: DMA + TensorE only."""
                p1 = p1s.tile([128, CB], BF16, tag="p1", name=f"p1_{blk}_{t}")
                (nc.sync if t % 2 == 0 else nc.scalar).dma_start(p1[:], xp1_d[blk, t])
                for ct in range(NCT):
                    nc.tensor.matmul(
                        z1[ct][:],
                        w0_sb[:, t * FN : (t + 1) * FN],
                        p1[:, ct * CT : (ct + 1) * CT],
                        start=(t == 0),
                        stop=(t == L1_CHUNKS - 1),
                    )

            def emit_h1_copy_and_abuild(blk, z1):
                """z1 -> h1_sb (bf16), the layer-2 H-tile build, and the
                out1 d-reduce (on gpsimd, keeping VectorE free)."""
                h1_sb = h1p.tile([128, CB], BF16, tag="h1", name=f"h1_{blk}")
                for ct in range(NCT):
                    nc.scalar.copy(
                        h1_sb[:, ct * CT : (ct + 1) * CT], z1[ct][:]
                    )
                a_sb = apool.tile([128, NSL * CB], BF16, tag="a", name=f"a_{blk}")
                for s in range(NSL):
                    (nc.sync if s % 2 == 0 else nc.scalar).dma_start(
                        a_sb[0:16, s * CB : (s + 1) * CB],
                        h1_sb[16 * s : 16 * s + 16, :],
                    )
                nc.sync.dma_start(a_sb[16:32, :], a_sb[0:16, :])
                nc.sync.dma_start(a_sb[32:64, :], a_sb[0:32, :])
                nc.scalar.dma_start(a_sb[64:96, :], a_sb[0:32, :])
                nc.scalar.dma_start(a_sb[96:128, :], a_sb[0:32, :])
                bo = blk * (CB // D)
                nc.vector.reduce_sum(
                    out_sb[:, bo : bo + CB // D],
                    h1_sb[:].rearrange("p (b d) -> p b d", d=D),
                    axis=mybir.AxisListType.X,
                )
                nc.sync.dma_start(
                    out_d[0][:, bo : bo + CB // D], out_sb[:, bo : bo + CB // D]
                )
                return a_sb

            # ---- layer 1 of block 0 (DMA-fed, no VectorE) ----
            z1_cur = alloc_z(1, 0)
            for t in range(L1_CHUNKS):
                emit_l1_step(0, z1_cur, t)
            a_cur = emit_h1_copy_and_abuild(0, z1_cur)

            xd_tiles = {}

            g2t_tiles = {}
            for blk in range(NBLK):
                c0 = blk * CB
                half_idx = blk // NBH
                if blk % NBH == 0:
                    g2t_tiles[half_idx] = l3sb.tile(
                        [128, NBH * NGB * 512],
                        BF16,
                        tag="g2t",
                        name=f"g2t_{half_idx}",
                    )
                a_sb = a_cur
                xb_sb = xb_tiles[blk]

                # ---------------- layer 2 over this block ----------------
                z2 = alloc_z(2, blk)
                if blk == 0:
                    # w2 deferred out of the fabric-bound init window; it is
                    # first needed by the o3 chain at the end of block 1
                    nc.gpsimd.dma_start(w2_sb[:], w2_d[:])
                # prefetch this block's layer-3 xdiag slab; it is consumed
                # one block later (or in the final block's own tail)
                xd_tiles[blk] = xdg.tile(
                    [128, NGB * 512], BF16, tag="xd", name=f"xd_{blk}"
                )
                nc.gpsimd.dma_start(
                    xd_tiles[blk][:],
                    xdiag_d[:, blk * NGB * 512 : (blk + 1) * NGB * 512],
                )
                if blk + 1 < NBLK:
                    xb_tiles[blk + 1] = xbp.tile(
                        [128, NGRP * CB], BF16, tag="xb", name=f"xb_{blk + 1}"
                    )
                    nc.gpsimd.dma_start(xb_tiles[blk + 1][:], xb_d[blk + 1])
                    z1_cur = alloc_z(1, blk + 1)

                l3blk = blk - 1
                # block 0's tail needs extra slack for the first H-tile build:
                # finish the next layer 1 a step earlier there
                l1_per_g = 4 if blk == 0 else 3
                ab_g = 4 if blk == 0 else 5
                for g in range(NGRP):
                    # interleave next block's layer 1 (front-loaded)
                    if blk + 1 < NBLK:
                        for t in range(l1_per_g * g,
                                       min(l1_per_g * (g + 1), L1_CHUNKS)):
                            emit_l1_step(blk + 1, z1_cur, t)
                    # one fused TT builds the 8 chunks (g, s) for this g
                    p_sb = pkr.tile(
                        [128, NSL * CB], BF16, tag="p", name=f"p2_{blk}_{g}"
                    )
                    xg = xb_sb[:, g * CB : (g + 1) * CB]
                    nc.vector.tensor_mul(
                        p_sb[:].rearrange("p (s c) -> p s c", s=NSL),
                        a_sb[:].rearrange("p (s c) -> p s c", s=NSL),
                        xg.unsqueeze(1).broadcast_to((128, NSL, CB)),
                    )
                    for s in range(NSL):
                        k = g * NSL + s
                        for ct in range(NCT):
                            nc.tensor.matmul(
                                z2[ct][:],
                                w1_sb[:, k * FN : (k + 1) * FN],
                                p_sb[:, s * CB + ct * CT : s * CB + (ct + 1) * CT],
                                start=(k == 0),
                                stop=(k == NGRP * NSL - 1),
                            )
                    # next block's h1/H-tile as soon as its layer 1 is done
                    if blk + 1 < NBLK and g == ab_g:
                        a_cur = emit_h1_copy_and_abuild(blk + 1, z1_cur)
                    # interleave layer 3 of the previous block (8 groups)
                    if l3blk >= 0:
                        gl = g
                        hidx = l3blk // NBH
                        g2t_sb = g2t_tiles[hidx]
                        gg = l3blk * NGB + gl
                        gh = (l3blk % NBH) * NGB + gl
                        ht_ps = l3ps.tile(
                            [128, 128], BF16, tag="l3", name=f"htps_{gg}"
                        )
                        nc.tensor.transpose(
                            ht_ps[:],
                            h2_sb[:, gg * 128 : (gg + 1) * 128],
                            ident_sb[:],
                        )
                        ht_sb = hts.tile(
                            [128, 128], BF16, tag="hts", name=f"htsb_{gg}"
                        )
                        nc.scalar.copy(ht_sb[:], ht_ps[:])
                        xd_sb = xd_tiles[l3blk][:, gl * 512 : (gl + 1) * 512]
                        g2_ps = l3ps.tile(
                            [128, 512], F32, tag="l3", name=f"g2ps_{gg}"
                        )
                        nc.tensor.matmul(g2_ps[:], ht_sb[:], xd_sb)
                        nc.scalar.copy(
                            g2t_sb[:, gh * 512 : (gh + 1) * 512], g2_ps[:]
                        )

                # z2 copy-out + out2 reduce (gpsimd) + drain
                for ct in range(NCT):
                    cc = c0 + ct * CT
                    nc.scalar.copy(h2_sb[:, cc : cc + CT], z2[ct][:])
                bo = blk * (CB // D)
                nc.vector.reduce_sum(
                    out_sb[:, BL + bo : BL + bo + CB // D],
                    h2_sb[:, c0 : c0 + CB].rearrange("p (b d) -> p b d", d=D),
                    axis=mybir.AxisListType.X,
                )
                nc.sync.dma_start(
                    out_d[1][:, bo : bo + CB // D],
                    out_sb[:, BL + bo : BL + bo + CB // D],
                )

                # ---- layer-3 contraction pieces (delayed one block) ----
                l3list = [blk - 1] if blk > 0 else []
                if blk == NBLK - 1:
                    l3list.append(blk)
                for l3blk2 in l3list:
                    hidx = l3blk2 // NBH
                    bi = l3blk2 % NBH
                    g2t_sb = g2t_tiles[hidx]
                    if l3blk2 == blk:
                        # emit the last block's 8 L3 groups (no next block
                        # to interleave them into)
                        for gl in range(NGB):
                            gg = l3blk2 * NGB + gl
                            gh = bi * NGB + gl
                            ht_ps = l3ps.tile(
                                [128, 128], BF16, tag="l3", name=f"htps_{gg}"
                            )
                            nc.tensor.transpose(
                                ht_ps[:],
                                h2_sb[:, gg * 128 : (gg + 1) * 128],
                                ident_sb[:],
                            )
                            ht_sb = hts.tile(
                                [128, 128], BF16, tag="hts", name=f"htsb_{gg}"
                            )
                            nc.scalar.copy(ht_sb[:], ht_ps[:])
                            xd_sb = xd_tiles[l3blk2][:, gl * 512 : (gl + 1) * 512]
                            g2_ps = l3ps.tile(
                                [128, 512], F32, tag="l3", name=f"g2ps_{gg}"
                            )
                            nc.tensor.matmul(g2_ps[:], ht_sb[:], xd_sb)
                            nc.scalar.copy(
                                g2t_sb[:, gh * 512 : (gh + 1) * 512], g2_ps[:]
                            )
                    g2t_r = g2t_sb[:].rearrange(
                        "p (g b i) -> p g b i", b=8, i=F0
                    )
                    if hidx == 1:
                        # last half: contract per block piece (N=64) so the
                        # first piece overlaps the final block's layer 2
                        if bi == 0:
                            o3_last = o3p.tile(
                                [128, 128], F32, tag="o3", name="o3_1"
                            )
                        for i in range(F0):
                            nc.tensor.matmul(
                                o3_last[:, bi * 64 : (bi + 1) * 64],
                                w2_sb[:, i * FN : (i + 1) * FN],
                                g2t_r[:, bi * NGB : (bi + 1) * NGB, :, i],
                                start=(i == 0),
                                stop=(i == F0 - 1),
                            )
                        o3_ps = o3_last
                    elif bi == NBH - 1:
                        # first half: one N=128 chain (fully overlapped)
                        o3_ps = o3p.tile(
                            [128, 128], F32, tag="o3", name=f"o3_{hidx}"
                        )
                        for i in range(F0):
                            nc.tensor.matmul(
                                o3_ps[:],
                                w2_sb[:, i * FN : (i + 1) * FN],
                                g2t_r[:, :, :, i],
                                start=(i == 0),
                                stop=(i == F0 - 1),
                            )
                    if bi == NBH - 1:
                        nc.scalar.copy(
                            out_sb[:, 2 * BL + hidx * 128 : 2 * BL + (hidx + 1) * 128],
                            o3_ps[:],
                        )
                        nc.sync.dma_start(
                            out_d[2][:, hidx * 128 : (hidx + 1) * 128],
                            out_sb[:, 2 * BL + hidx * 128 : 2 * BL + (hidx + 1) * 128],
                        )

    nc.finalize()
    return nc


def _prep_inputs(x, W0, W1, W2):
    """Host-side prep: shard x over cores, transpose/cast, chunk weights,
    build the layer-1 Khatri-Rao product and the layer-2 X-factor tiles."""
    bf = ml_dtypes.bfloat16
    xs = np.ascontiguousarray(x).reshape(NCORES, BL, F0, D)

    # symmetrized layer-1 weights: each unordered pair (i<=j) once, with
    # W0sym[(i,j)] = W0[i*64+j] + W0[j*64+i] (i<j); padded to 17*128 rows
    pi, pj = np.triu_indices(F0)                     # 2080 pairs, i <= j
    W0sym = np.zeros((L1_CHUNKS * 128, FN), dtype=np.float32)
    W0sym[:SYM_PAIRS] = W0[pi * F0 + pj]
    off = W0[pj * F0 + pi].copy()
    off[pi == pj] = 0.0
    W0sym[:SYM_PAIRS] += off
    w0c = (
        W0sym.reshape(L1_CHUNKS, 128, FN)
        .transpose(1, 0, 2)
        .reshape(128, L1_CHUNKS * FN)
    )
    w0c = np.ascontiguousarray(w0c).astype(bf)
    w2c = (
        W2.reshape(F0, 128, FN).transpose(1, 0, 2).reshape(128, F0 * FN)
    )
    w2c = np.ascontiguousarray(w2c).astype(bf)

    # layer-2 chunk (g, s), partition p -> i = 8g + p//16, j = 16s + p%16
    # w1c[p, (g*8+s)*FN + n] = W1[i*128 + j, n]
    W1r = W1.reshape(F0, FN, FN)                     # [i, j, n]
    p_ar = np.arange(128)
    w1c = np.empty((128, NGRP * NSL * FN), dtype=bf)
    for g in range(NGRP):
        for s in range(NSL):
            k = g * NSL + s
            w1c[:, k * FN : (k + 1) * FN] = W1r[
                8 * g + p_ar // 16, 16 * s + p_ar % 16
            ].astype(bf)
    ident = np.eye(128, dtype=np.float32).astype(bf)

    # row -> (i, j) map for the symmetrized layer-1 KR product
    i_idx = np.zeros(L1_CHUNKS * 128, dtype=np.int64)
    j_idx = np.zeros(L1_CHUNKS * 128, dtype=np.int64)
    i_idx[:SYM_PAIRS] = pi
    j_idx[:SYM_PAIRS] = pj

    in_maps = []
    for c in range(NCORES):
        xc = xs[c]                                   # [BL, F0, D]
        xt = xc.transpose(1, 0, 2).reshape(F0, C)    # [i, (b d)]
        xt_bf = xt.astype(bf)
        xt32 = xt_bf.astype(np.float32)

        # host-built layer-1 KR product, bf16-rounded like the device TT
        p1 = (xt32[i_idx] * xt32[j_idx]).astype(bf)  # [17*128, C]
        xp1 = (
            p1.reshape(L1_CHUNKS, 128, NBLK, CB)
            .transpose(2, 0, 1, 3)                   # [blk, t, 128, cb]
            .copy()
        )

        # layer-2 X-factor tiles: xbt[blk, p, g*CB + c] = X[8g + p//16, c]
        xtb = xt_bf.reshape(F0, NBLK, CB)            # [i, blk, cb]
        xbt = (
            xtb.reshape(NGRP, 8, NBLK, CB)[:, np.newaxis, :, :, :]  # g 1 r blk cb
            .repeat(16, axis=1)                      # g dup r blk cb
            .transpose(3, 2, 1, 0, 4)                # blk r dup g cb
            .reshape(NBLK, 128, NGRP * CB)
        )

        # xdiag[(bl', d), (g, bl, i)] = x[g*8+bl, i, d] if bl' == bl else 0
        xd = np.zeros((8, D, NG, 8, F0), dtype=bf)
        xg = xc.reshape(NG, 8, F0, D)                # [g, bl, i, d]
        for bl in range(8):
            xd[bl, :, :, bl, :] = xg[:, bl].transpose(2, 0, 1).astype(bf)
        xdiag = xd.reshape(128, NG * 512)

        in_maps.append(
            {
                "xp1": np.ascontiguousarray(xp1),
                "xbt": np.ascontiguousarray(xbt),
                "xdiag": np.ascontiguousarray(xdiag),
                "w0c": w0c,
                "w1c": np.ascontiguousarray(w1c),
                "w2c": w2c,
                "ident": ident,
            }
        )
    return in_maps


def _postprocess(results):
    # out_nb [3, 128 n, 256 b] per core -> [B, 384]
    outs = [
        np.asarray(r["out_nb"]).transpose(2, 0, 1).reshape(BL, 3 * FN)
        for r in results
    ]
    return np.ascontiguousarray(np.concatenate(outs, axis=0)).astype(np.float32)


def kernel(x, W0, W1, W2, _trace=False, _trace_kwargs=None):
    if "nc" not in _CACHE:
        _CACHE["nc"] = _build_program()
    nc = _CACHE["nc"]
    in_maps = _prep_inputs(
        np.asarray(x, dtype=np.float32),
        np.asarray(W0, dtype=np.float32),
        np.asarray(W1, dtype=np.float32),
        np.asarray(W2, dtype=np.float32),
    )
    kw = {}
    if _trace:
        kw["trace"] = True
        kw.update(_trace_kwargs or {})
    res = run_bass_kernel_spmd(nc, in_maps, core_ids=list(range(NCORES)), **kw)
    out = _postprocess(res.results)
    if _trace:
        _CACHE["last_results"] = res
    return out
